# revision 18
# baseline (speedup 1.0000x reference)
"""Multi-head causal attention (B=2, T=2048, D=1024, H=16) on 8 trn2 NeuronCores.

Sharding: data-parallel over batch (2) x tensor-parallel over heads (4 groups of
4 heads). Core c handles batch c//4, head group c%4. Each core computes its
heads' attention and a partial output projection; the host sums the 4 partials
per batch and adds b_out.

Schedule: the exp-paced attention inner loop is interleaved (by the Tile
scheduler's ready-heap) with next-chunk QKV and previous-chunk out-projection
matmuls so the PE array stays dense and HAM-warm. Scores for the two heads of
a pair land in one 2-bank PSUM tile so each exp activation covers 1024 cols.
"""

import sys

sys.path.insert(0, "/opt/trn_rl_repo")

import ml_dtypes
import numpy as np

import concourse.bass as bass
import concourse.tile as tile
from concourse import bacc, mybir
from concourse.bass_utils import run_bass_kernel_spmd

F32 = mybir.dt.float32
F32R = mybir.dt.float32r
BF16 = mybir.dt.bfloat16
FP8 = mybir.dt.float8e4

B, T, D, H = 2, 2048, 1024, 16
DH = D // H            # 64
HG = 4                 # heads per core
GCOLS = HG * DH        # 256 columns of qkv per core
NKT = T // 128         # 16 k-tiles of 128
NQC = T // 512         # 4 q-chunks of 512
NDT = D // 128         # 8 d-tiles of 128 (contraction)

_CACHED = {}


def _build():
    nc = bacc.Bacc("TRN2", target_bir_lowering=False, debug=False, num_devices=8)

    # host pre-layouts: xt[p, tch, a, c] = x[tch*512+c, a*128+p]
    #                   w[p, jt, a, c]  = wqkv_local[a*128+p, jt*128+c]
    xd = nc.dram_tensor("xt", [128, NQC, NDT, 512], BF16, kind="ExternalInput").ap()
    wd = nc.dram_tensor("w", [128, 6, NDT, 128], BF16, kind="ExternalInput").ap()
    bqpd = nc.dram_tensor("bqp", [128, 4], F32, kind="ExternalInput").ap()
    bvbd = nc.dram_tensor("bvb", [128, GCOLS], F32, kind="ExternalInput").ap()
    woutd = nc.dram_tensor("wout", [GCOLS, D], F32R, kind="ExternalInput").ap()
    maskd = nc.dram_tensor("mask2", [128, 256], F32, kind="ExternalInput").ap()
    outd = nc.dram_tensor("out", [T, D], BF16, kind="ExternalOutput").ap()

    Exp = mybir.ActivationFunctionType.Exp

    with tile.TileContext(nc) as tc:
        with tc.tile_pool(name="persist", bufs=1) as P, \
             tc.tile_pool(name="ps_s", bufs=2, space=bass.MemorySpace.PSUM) as ps_s, \
             tc.tile_pool(name="ps_o", bufs=2, space=bass.MemorySpace.PSUM) as ps_o, \
             tc.tile_pool(name="ps_k", bufs=2, space=bass.MemorySpace.PSUM) as ps_k, \
             tc.tile_pool(name="ppool", bufs=8) as ppool, \
             tc.tile_pool(name="rpool", bufs=3) as rpool, \
             tc.tile_pool(name="opool", bufs=3) as opool:

            # ---- small consts on the vector queue ----
            mask2 = P.tile([128, 2, 128], F32)
            nc.scalar.dma_start(out=mask2, in_=maskd.rearrange("p (h c) -> p h c", c=128))
            bqp_sb = P.tile([128, 4], F32)
            nc.scalar.dma_start(out=bqp_sb, in_=bqpd[:, :])
            bvb_sb = P.tile([128, HG, DH], F32)
            nc.scalar.dma_start(out=bvb_sb, in_=bvbd.rearrange("p (h c) -> p h c", c=DH))
            wout_sb = P.tile([128, 2, D], F32R)
            for i in range(2):
                nc.scalar.dma_start(out=wout_sb[:, i, :], in_=woutd[i * 128:(i + 1) * 128, :])

            # ---- bulk loads on the sync queue (issue order = need order) ----
            xt_sb = P.tile([128, NQC, NDT, 512], BF16)
            w_sb = P.tile([128, 6, NDT, 128], BF16)
            nc.sync.dma_start(out=w_sb[:, 0:4, :, :], in_=wd[:, 0:4, :, :])
            nc.sync.dma_start(out=xt_sb[:, 0, 0:4, :], in_=xd[:, 0, 0:4, :])
            nc.sync.dma_start(out=xt_sb[:, 0, 4:8, :], in_=xd[:, 0, 4:8, :])
            nc.sync.dma_start(out=w_sb[:, 4:6, :, :], in_=wd[:, 4:6, :, :])
            for tch in range(1, NQC):
                nc.sync.dma_start(out=xt_sb[:, tch, :, :], in_=xd[:, tch, :, :])

            # persistent activations
            qt = [P.tile([128, T], BF16, name=f"qt{p}") for p in range(2)]
            kt = [P.tile([128, T], BF16, name=f"kt{p}") for p in range(2)]
            v_aug = P.tile([128, NKT, HG * 65], BF16)
            vv = v_aug.rearrange("p k (h c) -> p k h c", c=65)
            nc.gpsimd.memset(v_aug, 1.0)  # col 64 of each 65-block stays 1.0
            ot = [P.tile([128, T], F32R, name=f"ot{p}") for p in range(2)]

            # HAM warm-up primer: dependency-free back-to-back matmuls that
            # run during the DMA/queue setup dead-zone so the PE clock is at
            # 2.4 GHz before real work starts.
            wtile = P.tile([128, 512], BF16)
            nc.vector.memset(wtile, 0.0)
            wps = ps_k.tile([128, 512], F32, tag="w", name="warm")
            for i in range(20):
                nc.tensor.matmul(
                    wps, wtile[:, 0:128], wtile, start=(i == 0), stop=(i == 19)
                )

            def qkv_chunk(qc):
                qs = slice(qc * 512, (qc + 1) * 512)
                for jt in (0, 2, 1, 3):
                    ps = ps_k.tile([128, 512], F32, tag="w", name=f"qk_{jt}_{qc}")
                    for a in range(NDT):
                        nc.tensor.matmul(
                            ps,
                            w_sb[:, jt, a, :],
                            xt_sb[:, qc, a, :],
                            start=(a == 0),
                            stop=(a == NDT - 1),
                        )
                    dst = (qt if jt < 2 else kt)[jt % 2]
                    nc.vector.tensor_scalar_add(dst[:, qs], ps, bqp_sb[:, jt:jt + 1])
                for k4 in range(4):
                    k = qc * 4 + k4
                    psv = ps_k.tile([128, GCOLS], F32, tag="w", name=f"v_{k}")
                    for a in range(NDT):
                        nc.tensor.matmul(
                            psv,
                            xt_sb[:, qc, a, k4 * 128:(k4 + 1) * 128],
                            w_sb[:, 4:6, a, :],
                            start=(a == 0),
                            stop=(a == NDT - 1),
                        )
                    nc.vector.tensor_add(
                        vv[:, k, :, 0:DH],
                        psv.rearrange("p (h c) -> p h c", c=DH),
                        bvb_sb,
                    )

            def att_chunk(qc):
                qs = slice(qc * 512, (qc + 1) * 512)
                n_kt = 4 * qc + 4
                if qc >= 1:
                    # warm-keeper gated on this chunk's qt: becomes ready at
                    # the window boundary and fills the observed ~1us gap
                    # (which otherwise also drops the HAM clock)
                    wk = ps_k.tile([128, 512], F32, tag="w", name=f"wkb_{qc}")
                    for i in range(3):
                        nc.tensor.matmul(
                            wk, wtile[:, 0:128], qt[0][:, qs],
                            start=(i == 0), stop=(i == 2),
                        )
                p_ref = None
                for pr in range(2):
                    o_ps = [
                        ps_o.tile([65, 512], F32, tag="o", name=f"o_{qc}_{pr}_{hh}")
                        for hh in range(2)
                    ]
                    for k in range(n_kt):
                        j = k - 4 * qc
                        c0 = j * 128 if j >= 0 else 0
                        s = ps_s.tile([128, 2, 512], F32, tag="s", name=f"s_{qc}_{pr}_{k}")
                        for hh in range(2):
                            half = slice(hh * 64, hh * 64 + 64)
                            nc.tensor.matmul(
                                s[:, hh, c0:512],
                                kt[pr][half, k * 128:(k + 1) * 128],
                                qt[pr][half, qc * 512 + c0:(qc + 1) * 512],
                                start=True,
                                stop=True,
                            )
                        if j >= 0:
                            nc.vector.tensor_add(
                                s[:, :, j * 128:(j + 1) * 128],
                                s[:, :, j * 128:(j + 1) * 128],
                                mask2,
                            )
                        p = ppool.tile([128, 2, 512], BF16, tag="p")
                        nc.scalar.activation(
                            p[:, :, c0:512], s[:, :, c0:512], Exp, scale=0.125
                        )
                        if qc == NQC - 1 and pr == 1 and k == 10:
                            p_ref = p
                        for hh in range(2):
                            nc.tensor.matmul(
                                o_ps[hh][:, c0:512],
                                vv[:, k, 2 * pr + hh, :],
                                p[:, hh, c0:512],
                                start=(k == 0),
                                stop=(k == n_kt - 1),
                            )
                    # normalization: denominator sits in row 64 of each o tile
                    for hh in range(2):
                        half = slice(hh * 64, hh * 64 + 64)
                        rr = rpool.tile([1, 512], F32, tag="rr", name=f"rr_{qc}_{pr}_{hh}")
                        nc.vector.tensor_copy(rr, o_ps[hh][64:65, :])
                        nc.vector.reciprocal_approx_fast(out=rr, in_=rr)
                        r2 = rpool.tile([64, 512], F32, tag="r2", name=f"r2_{qc}_{pr}_{hh}")
                        nc.gpsimd.partition_broadcast(r2, rr, channels=64)
                        if qc == NQC - 1:
                            # two pieces so the tail projection can start
                            # before the whole chunk is normalized
                            for i in range(2):
                                cs = slice(qc * 512 + i * 256, qc * 512 + (i + 1) * 256)
                                nc.vector.tensor_mul(
                                    ot[pr][half, cs],
                                    o_ps[hh][0:64, i * 256:(i + 1) * 256],
                                    r2[:, i * 256:(i + 1) * 256],
                                )
                        else:
                            nc.vector.tensor_mul(ot[pr][half, qs], o_ps[hh][0:64, :], r2)
                return p_ref

            def proj_chunk(qc):
                for tt in range(qc * 4, qc * 4 + 4):
                    ob = opool.tile([128, 2, 512], BF16, tag="ob")
                    for dc in range(2):
                        ps = ps_k.tile([128, 512], F32, tag="w", name=f"pr_{tt}_{dc}")
                        nc.tensor.matmul(
                            ps,
                            ot[0][:, tt * 128:(tt + 1) * 128],
                            wout_sb[:, 0, dc * 512:(dc + 1) * 512],
                            start=True,
                            stop=False,
                        )
                        nc.tensor.matmul(
                            ps,
                            ot[1][:, tt * 128:(tt + 1) * 128],
                            wout_sb[:, 1, dc * 512:(dc + 1) * 512],
                            start=False,
                            stop=True,
                        )
                        nc.scalar.copy(ob[:, dc, :], ps)
                    nc.sync.dma_start(
                        out=outd[tt * 128:(tt + 1) * 128, :],
                        in_=ob.rearrange("p a c -> p (a c)"),
                    )

            def tail_keeper(p_ref):
                # runs right after the last PV while the final normalization
                # chain drains on DVE/gpsimd, keeping the clock warm into the
                # tail projection
                wk = ps_k.tile([128, 512], F32, tag="w", name="wkt")
                for i in range(8):
                    nc.tensor.matmul(
                        wk, wtile[:, 0:128], p_ref[:, i % 2, :],
                        start=(i == 0), stop=(i == 7),
                    )

            # emission order = program order = scheduler priority among ready
            # instructions. att(qc) right after its qkv(qc); the next chunk's
            # QKV and all projections are emitted later, so the ready-heap
            # pulls them into exp-stall gaps on the PE instead of draining
            # them eagerly — the projections are the filler reserve for the
            # last (biggest) attention window.
            last_p = None
            for qc in range(NQC):
                qkv_chunk(qc)
                last_p = att_chunk(qc)
            tail_keeper(last_p)
            for qc in range(NQC):
                proj_chunk(qc)

    nc.compile()
    return nc


def _mask_np():
    rk = np.arange(128)[:, None]
    cq = np.arange(128)[None, :]
    return np.where(rk <= cq, 0.0, -1.0e30).astype(np.float32)


def _in_maps(x, w_qkv, b_qkv, w_out):
    bf16 = ml_dtypes.bfloat16
    mask = _mask_np()
    mask2 = np.ascontiguousarray(np.concatenate([mask, mask], axis=1))
    in_maps = []
    for c in range(8):
        b, g = divmod(c, 4)
        cols = slice(g * GCOLS, (g + 1) * GCOLS)
        wq = np.concatenate(
            [w_qkv[:, :D][:, cols], w_qkv[:, D:2 * D][:, cols], w_qkv[:, 2 * D:][:, cols]],
            axis=1,
        )
        bq = np.concatenate([b_qkv[:D][cols], b_qkv[D:2 * D][cols], b_qkv[2 * D:][cols]])
        whost = wq.reshape(NDT, 128, 6, 128).transpose(1, 2, 0, 3)
        xhost = x[b].reshape(NQC, 512, NDT, 128).transpose(3, 0, 2, 1)
        in_maps.append(
            {
                "xt": np.ascontiguousarray(xhost.astype(bf16)),
                "w": np.ascontiguousarray(whost.astype(bf16)),
                "bqp": np.ascontiguousarray(
                    bq[:2 * GCOLS].reshape(4, 128).T.astype(np.float32)
                ),
                "bvb": np.ascontiguousarray(
                    np.broadcast_to(bq[2 * GCOLS:], (128, GCOLS)).astype(np.float32)
                ),
                "wout": np.ascontiguousarray(w_out[cols, :].astype(np.float32)),
                "mask2": mask2,
            }
        )
    return in_maps


def kernel(x, w_qkv, b_qkv, w_out, b_out):
    x = np.ascontiguousarray(np.asarray(x, dtype=np.float32))
    w_qkv = np.ascontiguousarray(np.asarray(w_qkv, dtype=np.float32))
    b_qkv = np.asarray(b_qkv, dtype=np.float32)
    w_out = np.ascontiguousarray(np.asarray(w_out, dtype=np.float32))
    b_out = np.asarray(b_out, dtype=np.float32)

    if "nc" not in _CACHED:
        _CACHED["nc"] = _build()
    nc = _CACHED["nc"]

    res = run_bass_kernel_spmd(nc, _in_maps(x, w_qkv, b_qkv, w_out), list(range(8)))
    out = np.zeros((B, T, D), dtype=np.float32)
    for c in range(8):
        out[c // 4] += res.results[c]["out"].astype(np.float32)
    out += b_out
    return out


# revision 19
# speedup vs baseline: 1.0608x; 1.0608x over previous
"""Multi-head causal attention (B=2, T=2048, D=1024, H=16) on 8 trn2 NeuronCores.

Sharding: data-parallel over batch (2) x tensor-parallel over heads (4 groups of
4 heads). Core c handles batch c//4, head group c%4. Each core computes its
heads' attention and a partial output projection; the host sums the 4 partials
per batch and adds b_out.

Schedule: the exp-paced attention inner loop is interleaved (by the Tile
scheduler's ready-heap) with next-chunk QKV and previous-chunk out-projection
matmuls so the PE array stays dense and HAM-warm. Scores for the two heads of
a pair land in one 2-bank PSUM tile so each exp activation covers 1024 cols.
"""

import sys

sys.path.insert(0, "/opt/trn_rl_repo")

import ml_dtypes
import numpy as np

import concourse.bass as bass
import concourse.tile as tile
from concourse import bacc, mybir
from concourse.bass_utils import run_bass_kernel_spmd

F32 = mybir.dt.float32
F32R = mybir.dt.float32r
BF16 = mybir.dt.bfloat16
FP8 = mybir.dt.float8e4

B, T, D, H = 2, 2048, 1024, 16
DH = D // H            # 64
HG = 4                 # heads per core
GCOLS = HG * DH        # 256 columns of qkv per core
NKT = T // 128         # 16 k-tiles of 128
NQC = T // 512         # 4 q-chunks of 512
NDT = D // 128         # 8 d-tiles of 128 (contraction)

_CACHED = {}


def _build():
    nc = bacc.Bacc("TRN2", target_bir_lowering=False, debug=False, num_devices=8)

    # host pre-layouts: xt[p, tch, a, c] = x[tch*512+c, a*128+p]
    #                   w[p, jt, a, c]  = wqkv_local[a*128+p, jt*128+c]
    xd = nc.dram_tensor("xt", [128, NQC, NDT, 512], BF16, kind="ExternalInput").ap()
    wd = nc.dram_tensor("w", [128, 6, NDT, 128], BF16, kind="ExternalInput").ap()
    bqpd = nc.dram_tensor("bqp", [128, 4], F32, kind="ExternalInput").ap()
    bvbd = nc.dram_tensor("bvb", [128, GCOLS], F32, kind="ExternalInput").ap()
    woutd = nc.dram_tensor("wout", [GCOLS, D], F32R, kind="ExternalInput").ap()
    maskd = nc.dram_tensor("mask2", [128, 256], F32, kind="ExternalInput").ap()
    outd = nc.dram_tensor("out", [T, D], BF16, kind="ExternalOutput").ap()

    Exp = mybir.ActivationFunctionType.Exp

    with tile.TileContext(nc) as tc:
        with tc.tile_pool(name="persist", bufs=1) as P, \
             tc.tile_pool(name="ps_s", bufs=2, space=bass.MemorySpace.PSUM) as ps_s, \
             tc.tile_pool(name="ps_o", bufs=2, space=bass.MemorySpace.PSUM) as ps_o, \
             tc.tile_pool(name="ps_k", bufs=2, space=bass.MemorySpace.PSUM) as ps_k, \
             tc.tile_pool(name="ppool", bufs=6) as ppool, \
             tc.tile_pool(name="rpool", bufs=2) as rpool, \
             tc.tile_pool(name="opool", bufs=3) as opool:

            # ---- small consts on the vector queue ----
            mask2 = P.tile([128, 2, 128], F32)
            nc.scalar.dma_start(out=mask2, in_=maskd.rearrange("p (h c) -> p h c", c=128))
            bqp_sb = P.tile([128, 4], F32)
            nc.scalar.dma_start(out=bqp_sb, in_=bqpd[:, :])
            bvb_sb = P.tile([128, HG, DH], F32)
            nc.scalar.dma_start(out=bvb_sb, in_=bvbd.rearrange("p (h c) -> p h c", c=DH))
            wout_sb = P.tile([128, 2, D], F32R)
            for i in range(2):
                nc.scalar.dma_start(out=wout_sb[:, i, :], in_=woutd[i * 128:(i + 1) * 128, :])

            # ---- bulk loads on the sync queue (issue order = need order) ----
            xt_sb = P.tile([128, NQC, NDT, 512], BF16)
            w_sb = P.tile([128, 6, NDT, 128], BF16)
            nc.sync.dma_start(out=w_sb[:, 0:4, :, :], in_=wd[:, 0:4, :, :])
            nc.sync.dma_start(out=xt_sb[:, 0, 0:4, :], in_=xd[:, 0, 0:4, :])
            nc.sync.dma_start(out=xt_sb[:, 0, 4:8, :], in_=xd[:, 0, 4:8, :])
            nc.sync.dma_start(out=w_sb[:, 4:6, :, :], in_=wd[:, 4:6, :, :])
            for tch in range(1, NQC):
                nc.sync.dma_start(out=xt_sb[:, tch, :, :], in_=xd[:, tch, :, :])

            # persistent activations
            qt = [P.tile([128, T], BF16, name=f"qt{p}") for p in range(2)]
            kt = [P.tile([128, T], BF16, name=f"kt{p}") for p in range(2)]
            v_aug = P.tile([128, NKT, HG * 65], BF16)
            vv = v_aug.rearrange("p k (h c) -> p k h c", c=65)
            nc.gpsimd.memset(v_aug, 1.0)  # col 64 of each 65-block stays 1.0
            ot = [P.tile([128, T], F32R, name=f"ot{p}") for p in range(2)]


            def qkv_chunk(qc):
                qs = slice(qc * 512, (qc + 1) * 512)
                for jt in range(4):
                    ps = ps_k.tile([128, 512], F32, tag="w", name=f"qk_{jt}_{qc}")
                    for a in range(NDT):
                        nc.tensor.matmul(
                            ps,
                            w_sb[:, jt, a, :],
                            xt_sb[:, qc, a, :],
                            start=(a == 0),
                            stop=(a == NDT - 1),
                        )
                    dst = qt[jt] if jt < 2 else kt[jt - 2]
                    nc.vector.tensor_scalar_add(dst[:, qs], ps, bqp_sb[:, jt:jt + 1])
                for k4 in range(4):
                    k = qc * 4 + k4
                    psv = ps_k.tile([128, GCOLS], F32, tag="w", name=f"v_{k}")
                    for a in range(NDT):
                        nc.tensor.matmul(
                            psv,
                            xt_sb[:, qc, a, k4 * 128:(k4 + 1) * 128],
                            w_sb[:, 4:6, a, :],
                            start=(a == 0),
                            stop=(a == NDT - 1),
                        )
                    nc.vector.tensor_add(
                        vv[:, k, :, 0:DH],
                        psv.rearrange("p (h c) -> p h c", c=DH),
                        bvb_sb,
                    )

            def att_chunk(qc):
                qs = slice(qc * 512, (qc + 1) * 512)
                n_kt = 4 * qc + 4
                for pr in range(2):
                    o_ps = [
                        ps_o.tile([65, 512], F32, tag="o", name=f"o_{qc}_{pr}_{hh}")
                        for hh in range(2)
                    ]
                    for k in range(n_kt):
                        j = k - 4 * qc
                        c0 = j * 128 if j >= 0 else 0
                        s = ps_s.tile([128, 2, 512], F32, tag="s", name=f"s_{qc}_{pr}_{k}")
                        for hh in range(2):
                            half = slice(hh * 64, hh * 64 + 64)
                            nc.tensor.matmul(
                                s[:, hh, c0:512],
                                kt[pr][half, k * 128:(k + 1) * 128],
                                qt[pr][half, qc * 512 + c0:(qc + 1) * 512],
                                start=True,
                                stop=True,
                            )
                        if j >= 0:
                            nc.vector.tensor_add(
                                s[:, :, j * 128:(j + 1) * 128],
                                s[:, :, j * 128:(j + 1) * 128],
                                mask2,
                            )
                        p = ppool.tile([128, 2, 512], BF16, tag="p")
                        nc.scalar.activation(
                            p[:, :, c0:512], s[:, :, c0:512], Exp, scale=0.125
                        )
                        for hh in range(2):
                            nc.tensor.matmul(
                                o_ps[hh][:, c0:512],
                                vv[:, k, 2 * pr + hh, :],
                                p[:, hh, c0:512],
                                start=(k == 0),
                                stop=(k == n_kt - 1),
                            )
                    # normalization: denominator sits in row 64 of each o tile
                    for hh in range(2):
                        half = slice(hh * 64, hh * 64 + 64)
                        rr = rpool.tile([1, 512], F32, tag="rr", name=f"rr_{qc}_{pr}_{hh}")
                        nc.vector.tensor_copy(rr, o_ps[hh][64:65, :])
                        nc.vector.reciprocal_approx_fast(out=rr, in_=rr)
                        r2 = rpool.tile([64, 512], F32, tag="r2", name=f"r2_{qc}_{pr}_{hh}")
                        nc.gpsimd.partition_broadcast(r2, rr, channels=64)
                        nc.vector.tensor_mul(ot[pr][half, qs], o_ps[hh][0:64, :], r2)

            def proj_chunk(qc):
                for tt in range(qc * 4, qc * 4 + 4):
                    ob = opool.tile([128, 2, 512], BF16, tag="ob")
                    for dc in range(2):
                        ps = ps_k.tile([128, 512], F32, tag="w", name=f"pr_{tt}_{dc}")
                        nc.tensor.matmul(
                            ps,
                            ot[0][:, tt * 128:(tt + 1) * 128],
                            wout_sb[:, 0, dc * 512:(dc + 1) * 512],
                            start=True,
                            stop=False,
                        )
                        nc.tensor.matmul(
                            ps,
                            ot[1][:, tt * 128:(tt + 1) * 128],
                            wout_sb[:, 1, dc * 512:(dc + 1) * 512],
                            start=False,
                            stop=True,
                        )
                        nc.vector.tensor_copy(ob[:, dc, :], ps)
                    nc.sync.dma_start(
                        out=outd[tt * 128:(tt + 1) * 128, :],
                        in_=ob.rearrange("p a c -> p (a c)"),
                    )

            # emission order = program order = scheduler priority among ready
            # instructions. att(qc) right after its qkv(qc); the next chunk's
            # QKV and all projections are emitted later, so the ready-heap
            # pulls them into exp-stall gaps on the PE instead of draining
            # them eagerly — the projections are the filler reserve for the
            # last (biggest) attention window.
            for qc in range(NQC):
                qkv_chunk(qc)
                att_chunk(qc)
            for qc in range(NQC):
                proj_chunk(qc)

    nc.compile()
    return nc


def _mask_np():
    rk = np.arange(128)[:, None]
    cq = np.arange(128)[None, :]
    return np.where(rk <= cq, 0.0, -1.0e30).astype(np.float32)


def _in_maps(x, w_qkv, b_qkv, w_out):
    bf16 = ml_dtypes.bfloat16
    mask = _mask_np()
    mask2 = np.ascontiguousarray(np.concatenate([mask, mask], axis=1))
    in_maps = []
    for c in range(8):
        b, g = divmod(c, 4)
        cols = slice(g * GCOLS, (g + 1) * GCOLS)
        wq = np.concatenate(
            [w_qkv[:, :D][:, cols], w_qkv[:, D:2 * D][:, cols], w_qkv[:, 2 * D:][:, cols]],
            axis=1,
        )
        bq = np.concatenate([b_qkv[:D][cols], b_qkv[D:2 * D][cols], b_qkv[2 * D:][cols]])
        whost = wq.reshape(NDT, 128, 6, 128).transpose(1, 2, 0, 3)
        xhost = x[b].reshape(NQC, 512, NDT, 128).transpose(3, 0, 2, 1)
        in_maps.append(
            {
                "xt": np.ascontiguousarray(xhost.astype(bf16)),
                "w": np.ascontiguousarray(whost.astype(bf16)),
                "bqp": np.ascontiguousarray(
                    bq[:2 * GCOLS].reshape(4, 128).T.astype(np.float32)
                ),
                "bvb": np.ascontiguousarray(
                    np.broadcast_to(bq[2 * GCOLS:], (128, GCOLS)).astype(np.float32)
                ),
                "wout": np.ascontiguousarray(w_out[cols, :].astype(np.float32)),
                "mask2": mask2,
            }
        )
    return in_maps


def kernel(x, w_qkv, b_qkv, w_out, b_out):
    x = np.ascontiguousarray(np.asarray(x, dtype=np.float32))
    w_qkv = np.ascontiguousarray(np.asarray(w_qkv, dtype=np.float32))
    b_qkv = np.asarray(b_qkv, dtype=np.float32)
    w_out = np.ascontiguousarray(np.asarray(w_out, dtype=np.float32))
    b_out = np.asarray(b_out, dtype=np.float32)

    if "nc" not in _CACHED:
        _CACHED["nc"] = _build()
    nc = _CACHED["nc"]

    res = run_bass_kernel_spmd(nc, _in_maps(x, w_qkv, b_qkv, w_out), list(range(8)))
    out = np.zeros((B, T, D), dtype=np.float32)
    for c in range(8):
        out[c // 4] += res.results[c]["out"].astype(np.float32)
    out += b_out
    return out


# revision 20
# speedup vs baseline: 1.0793x; 1.0175x over previous
"""Multi-head causal attention (B=2, T=2048, D=1024, H=16) on 8 trn2 NeuronCores.

Sharding: data-parallel over batch (2) x tensor-parallel over heads (4 groups of
4 heads). Core c handles batch c//4, head group c%4. Each core computes its
heads' attention and a partial output projection; the host sums the 4 partials
per batch and adds b_out.

Schedule: the exp-paced attention inner loop is interleaved (by the Tile
scheduler's ready-heap) with next-chunk QKV and previous-chunk out-projection
matmuls so the PE array stays dense and HAM-warm. Scores for the two heads of
a pair land in one 2-bank PSUM tile so each exp activation covers 1024 cols.
"""

import sys

sys.path.insert(0, "/opt/trn_rl_repo")

import ml_dtypes
import numpy as np

import concourse.bass as bass
import concourse.tile as tile
from concourse import bacc, mybir
from concourse.bass_utils import run_bass_kernel_spmd

F32 = mybir.dt.float32
F32R = mybir.dt.float32r
BF16 = mybir.dt.bfloat16
FP8 = mybir.dt.float8e4

B, T, D, H = 2, 2048, 1024, 16
DH = D // H            # 64
HG = 4                 # heads per core
GCOLS = HG * DH        # 256 columns of qkv per core
NKT = T // 128         # 16 k-tiles of 128
NQC = T // 512         # 4 q-chunks of 512
NDT = D // 128         # 8 d-tiles of 128 (contraction)

_CACHED = {}


def _build():
    nc = bacc.Bacc("TRN2", target_bir_lowering=False, debug=False, num_devices=8)

    # host pre-layouts: xt[p, tch, a, c] = x[tch*512+c, a*128+p]
    #                   w[p, jt, a, c]  = wqkv_local[a*128+p, jt*128+c]
    xd = nc.dram_tensor("xt", [128, NQC, NDT, 512], BF16, kind="ExternalInput").ap()
    wd = nc.dram_tensor("w", [128, 6, NDT, 128], BF16, kind="ExternalInput").ap()
    bqpd = nc.dram_tensor("bqp", [128, 4], F32, kind="ExternalInput").ap()
    bvbd = nc.dram_tensor("bvb", [128, GCOLS], F32, kind="ExternalInput").ap()
    woutd = nc.dram_tensor("wout", [GCOLS, D], F32R, kind="ExternalInput").ap()
    maskd = nc.dram_tensor("mask2", [128, 256], F32, kind="ExternalInput").ap()
    outd = nc.dram_tensor("out", [T, D], BF16, kind="ExternalOutput").ap()

    Exp = mybir.ActivationFunctionType.Exp

    with tile.TileContext(nc) as tc:
        with tc.tile_pool(name="persist", bufs=1) as P, \
             tc.tile_pool(name="ps_s", bufs=2, space=bass.MemorySpace.PSUM) as ps_s, \
             tc.tile_pool(name="ps_o", bufs=2, space=bass.MemorySpace.PSUM) as ps_o, \
             tc.tile_pool(name="ps_k", bufs=2, space=bass.MemorySpace.PSUM) as ps_k, \
             tc.tile_pool(name="ppool", bufs=8) as ppool, \
             tc.tile_pool(name="rpool", bufs=3) as rpool, \
             tc.tile_pool(name="opool", bufs=3) as opool:

            # ---- small consts on the vector queue ----
            mask2 = P.tile([128, 2, 128], F32)
            nc.scalar.dma_start(out=mask2, in_=maskd.rearrange("p (h c) -> p h c", c=128))
            bqp_sb = P.tile([128, 4], F32)
            nc.scalar.dma_start(out=bqp_sb, in_=bqpd[:, :])
            bvb_sb = P.tile([128, HG, DH], F32)
            nc.scalar.dma_start(out=bvb_sb, in_=bvbd.rearrange("p (h c) -> p h c", c=DH))
            wout_sb = P.tile([128, 2, D], F32R)
            for i in range(2):
                nc.scalar.dma_start(out=wout_sb[:, i, :], in_=woutd[i * 128:(i + 1) * 128, :])

            # ---- bulk loads on the sync queue (issue order = need order) ----
            xt_sb = P.tile([128, NQC, NDT, 512], BF16)
            w_sb = P.tile([128, 6, NDT, 128], BF16)
            nc.sync.dma_start(out=w_sb[:, 0:4, :, :], in_=wd[:, 0:4, :, :])
            nc.sync.dma_start(out=xt_sb[:, 0, 0:4, :], in_=xd[:, 0, 0:4, :])
            nc.sync.dma_start(out=xt_sb[:, 0, 4:8, :], in_=xd[:, 0, 4:8, :])
            nc.sync.dma_start(out=w_sb[:, 4:6, :, :], in_=wd[:, 4:6, :, :])
            for tch in range(1, NQC):
                nc.sync.dma_start(out=xt_sb[:, tch, :, :], in_=xd[:, tch, :, :])

            # persistent activations
            qt = [P.tile([128, T], BF16, name=f"qt{p}") for p in range(2)]
            kt = [P.tile([128, T], BF16, name=f"kt{p}") for p in range(2)]
            v_aug = P.tile([128, NKT, HG * 65], BF16)
            vv = v_aug.rearrange("p k (h c) -> p k h c", c=65)
            nc.gpsimd.memset(v_aug, 1.0)  # col 64 of each 65-block stays 1.0
            ot = [P.tile([128, T], F32R, name=f"ot{p}") for p in range(2)]

            # HAM warm-up primer: dependency-free back-to-back matmuls that
            # run during the DMA/queue setup dead-zone so the PE clock is at
            # 2.4 GHz before real work starts.
            wtile = P.tile([128, 512], BF16)
            nc.vector.memset(wtile, 0.0)
            wps = ps_k.tile([128, 512], F32, tag="w", name="warm")
            for i in range(20):
                nc.tensor.matmul(
                    wps, wtile[:, 0:128], wtile, start=(i == 0), stop=(i == 19)
                )


            def qkv_chunk(qc):
                qs = slice(qc * 512, (qc + 1) * 512)
                for jt in (0, 2, 1, 3):
                    ps = ps_k.tile([128, 512], F32, tag="w", name=f"qk_{jt}_{qc}")
                    for a in range(NDT):
                        nc.tensor.matmul(
                            ps,
                            w_sb[:, jt, a, :],
                            xt_sb[:, qc, a, :],
                            start=(a == 0),
                            stop=(a == NDT - 1),
                        )
                    dst = (qt if jt < 2 else kt)[jt % 2]
                    nc.vector.tensor_scalar_add(dst[:, qs], ps, bqp_sb[:, jt:jt + 1])
                for k4 in range(4):
                    k = qc * 4 + k4
                    psv = ps_k.tile([128, GCOLS], F32, tag="w", name=f"v_{k}")
                    for a in range(NDT):
                        nc.tensor.matmul(
                            psv,
                            xt_sb[:, qc, a, k4 * 128:(k4 + 1) * 128],
                            w_sb[:, 4:6, a, :],
                            start=(a == 0),
                            stop=(a == NDT - 1),
                        )
                    nc.vector.tensor_add(
                        vv[:, k, :, 0:DH],
                        psv.rearrange("p (h c) -> p h c", c=DH),
                        bvb_sb,
                    )

            def att_chunk(qc):
                qs = slice(qc * 512, (qc + 1) * 512)
                n_kt = 4 * qc + 4
                for pr in range(2):
                    o_ps = [
                        ps_o.tile([65, 512], F32, tag="o", name=f"o_{qc}_{pr}_{hh}")
                        for hh in range(2)
                    ]
                    for k in range(n_kt):
                        j = k - 4 * qc
                        c0 = j * 128 if j >= 0 else 0
                        s = ps_s.tile([128, 2, 512], F32, tag="s", name=f"s_{qc}_{pr}_{k}")
                        for hh in range(2):
                            half = slice(hh * 64, hh * 64 + 64)
                            nc.tensor.matmul(
                                s[:, hh, c0:512],
                                kt[pr][half, k * 128:(k + 1) * 128],
                                qt[pr][half, qc * 512 + c0:(qc + 1) * 512],
                                start=True,
                                stop=True,
                            )
                        if j >= 0:
                            nc.vector.tensor_add(
                                s[:, :, j * 128:(j + 1) * 128],
                                s[:, :, j * 128:(j + 1) * 128],
                                mask2,
                            )
                        p = ppool.tile([128, 2, 512], BF16, tag="p")
                        nc.scalar.activation(
                            p[:, :, c0:512], s[:, :, c0:512], Exp, scale=0.125
                        )
                        for hh in range(2):
                            nc.tensor.matmul(
                                o_ps[hh][:, c0:512],
                                vv[:, k, 2 * pr + hh, :],
                                p[:, hh, c0:512],
                                start=(k == 0),
                                stop=(k == n_kt - 1),
                            )
                    # normalization: denominator sits in row 64 of each o tile
                    for hh in range(2):
                        half = slice(hh * 64, hh * 64 + 64)
                        rr = rpool.tile([1, 512], F32, tag="rr", name=f"rr_{qc}_{pr}_{hh}")
                        nc.vector.tensor_copy(rr, o_ps[hh][64:65, :])
                        nc.vector.reciprocal_approx_fast(out=rr, in_=rr)
                        r2 = rpool.tile([64, 512], F32, tag="r2", name=f"r2_{qc}_{pr}_{hh}")
                        nc.gpsimd.partition_broadcast(r2, rr, channels=64)
                        if qc == NQC - 1:
                            # two pieces so the tail projection can start
                            # before the whole chunk is normalized
                            for i in range(2):
                                cs = slice(qc * 512 + i * 256, qc * 512 + (i + 1) * 256)
                                nc.vector.tensor_mul(
                                    ot[pr][half, cs],
                                    o_ps[hh][0:64, i * 256:(i + 1) * 256],
                                    r2[:, i * 256:(i + 1) * 256],
                                )
                        else:
                            nc.vector.tensor_mul(ot[pr][half, qs], o_ps[hh][0:64, :], r2)

            def proj_chunk(qc):
                for tt in range(qc * 4, qc * 4 + 4):
                    ob = opool.tile([128, 2, 512], BF16, tag="ob")
                    for dc in range(2):
                        ps = ps_k.tile([128, 512], F32, tag="w", name=f"pr_{tt}_{dc}")
                        nc.tensor.matmul(
                            ps,
                            ot[0][:, tt * 128:(tt + 1) * 128],
                            wout_sb[:, 0, dc * 512:(dc + 1) * 512],
                            start=True,
                            stop=False,
                        )
                        nc.tensor.matmul(
                            ps,
                            ot[1][:, tt * 128:(tt + 1) * 128],
                            wout_sb[:, 1, dc * 512:(dc + 1) * 512],
                            start=False,
                            stop=True,
                        )
                        nc.scalar.copy(ob[:, dc, :], ps)
                    nc.sync.dma_start(
                        out=outd[tt * 128:(tt + 1) * 128, :],
                        in_=ob.rearrange("p a c -> p (a c)"),
                    )

            # emission order = program order = scheduler priority among ready
            # instructions. att(qc) right after its qkv(qc); the next chunk's
            # QKV and all projections are emitted later, so the ready-heap
            # pulls them into exp-stall gaps on the PE instead of draining
            # them eagerly — the projections are the filler reserve for the
            # last (biggest) attention window.
            for qc in range(NQC):
                qkv_chunk(qc)
                att_chunk(qc)
            for qc in range(NQC):
                proj_chunk(qc)

    nc.compile()
    return nc


def _mask_np():
    rk = np.arange(128)[:, None]
    cq = np.arange(128)[None, :]
    return np.where(rk <= cq, 0.0, -1.0e30).astype(np.float32)


def _in_maps(x, w_qkv, b_qkv, w_out):
    bf16 = ml_dtypes.bfloat16
    mask = _mask_np()
    mask2 = np.ascontiguousarray(np.concatenate([mask, mask], axis=1))
    in_maps = []
    for c in range(8):
        b, g = divmod(c, 4)
        cols = slice(g * GCOLS, (g + 1) * GCOLS)
        wq = np.concatenate(
            [w_qkv[:, :D][:, cols], w_qkv[:, D:2 * D][:, cols], w_qkv[:, 2 * D:][:, cols]],
            axis=1,
        )
        bq = np.concatenate([b_qkv[:D][cols], b_qkv[D:2 * D][cols], b_qkv[2 * D:][cols]])
        whost = wq.reshape(NDT, 128, 6, 128).transpose(1, 2, 0, 3)
        xhost = x[b].reshape(NQC, 512, NDT, 128).transpose(3, 0, 2, 1)
        in_maps.append(
            {
                "xt": np.ascontiguousarray(xhost.astype(bf16)),
                "w": np.ascontiguousarray(whost.astype(bf16)),
                "bqp": np.ascontiguousarray(
                    bq[:2 * GCOLS].reshape(4, 128).T.astype(np.float32)
                ),
                "bvb": np.ascontiguousarray(
                    np.broadcast_to(bq[2 * GCOLS:], (128, GCOLS)).astype(np.float32)
                ),
                "wout": np.ascontiguousarray(w_out[cols, :].astype(np.float32)),
                "mask2": mask2,
            }
        )
    return in_maps


def kernel(x, w_qkv, b_qkv, w_out, b_out):
    x = np.ascontiguousarray(np.asarray(x, dtype=np.float32))
    w_qkv = np.ascontiguousarray(np.asarray(w_qkv, dtype=np.float32))
    b_qkv = np.asarray(b_qkv, dtype=np.float32)
    w_out = np.ascontiguousarray(np.asarray(w_out, dtype=np.float32))
    b_out = np.asarray(b_out, dtype=np.float32)

    if "nc" not in _CACHED:
        _CACHED["nc"] = _build()
    nc = _CACHED["nc"]

    res = run_bass_kernel_spmd(nc, _in_maps(x, w_qkv, b_qkv, w_out), list(range(8)))
    out = np.zeros((B, T, D), dtype=np.float32)
    for c in range(8):
        out[c // 4] += res.results[c]["out"].astype(np.float32)
    out += b_out
    return out


# revision 21
# speedup vs baseline: 1.0844x; 1.0047x over previous
"""Multi-head causal attention (B=2, T=2048, D=1024, H=16) on 8 trn2 NeuronCores.

Sharding: data-parallel over batch (2) x tensor-parallel over heads (4 groups of
4 heads). Core c handles batch c//4, head group c%4. Each core computes its
heads' attention and a partial output projection; the host sums the 4 partials
per batch and adds b_out.

Schedule: the exp-paced attention inner loop is interleaved (by the Tile
scheduler's ready-heap) with next-chunk QKV and previous-chunk out-projection
matmuls so the PE array stays dense and HAM-warm. Scores for the two heads of
a pair land in one 2-bank PSUM tile so each exp activation covers 1024 cols.
"""

import sys

sys.path.insert(0, "/opt/trn_rl_repo")

import ml_dtypes
import numpy as np

import concourse.bass as bass
import concourse.tile as tile
from concourse import bacc, mybir
from concourse.bass_utils import run_bass_kernel_spmd

F32 = mybir.dt.float32
F32R = mybir.dt.float32r
BF16 = mybir.dt.bfloat16
FP8 = mybir.dt.float8e4

B, T, D, H = 2, 2048, 1024, 16
DH = D // H            # 64
HG = 4                 # heads per core
GCOLS = HG * DH        # 256 columns of qkv per core
NKT = T // 128         # 16 k-tiles of 128
NQC = T // 512         # 4 q-chunks of 512
NDT = D // 128         # 8 d-tiles of 128 (contraction)

_CACHED = {}


def _build():
    nc = bacc.Bacc("TRN2", target_bir_lowering=False, debug=False, num_devices=8)

    # host pre-layouts: xt[p, tch, a, c] = x[tch*512+c, a*128+p]
    #                   w[p, jt, a, c]  = wqkv_local[a*128+p, jt*128+c]
    xd = nc.dram_tensor("xt", [128, NQC, NDT, 512], BF16, kind="ExternalInput").ap()
    wd = nc.dram_tensor("w", [128, 6, NDT, 128], BF16, kind="ExternalInput").ap()
    bqpd = nc.dram_tensor("bqp", [128, 4], F32, kind="ExternalInput").ap()
    bvbd = nc.dram_tensor("bvb", [128, GCOLS], F32, kind="ExternalInput").ap()
    woutd = nc.dram_tensor("wout", [GCOLS, D], F32R, kind="ExternalInput").ap()
    maskd = nc.dram_tensor("mask2", [128, 256], F32, kind="ExternalInput").ap()
    outd = nc.dram_tensor("out", [T, D], BF16, kind="ExternalOutput").ap()

    Exp = mybir.ActivationFunctionType.Exp

    with tile.TileContext(nc) as tc:
        with tc.tile_pool(name="persist", bufs=1) as P, \
             tc.tile_pool(name="ps_s", bufs=2, space=bass.MemorySpace.PSUM) as ps_s, \
             tc.tile_pool(name="ps_o", bufs=2, space=bass.MemorySpace.PSUM) as ps_o, \
             tc.tile_pool(name="ps_k", bufs=2, space=bass.MemorySpace.PSUM) as ps_k, \
             tc.tile_pool(name="ppool", bufs=8) as ppool, \
             tc.tile_pool(name="rpool", bufs=3) as rpool, \
             tc.tile_pool(name="opool", bufs=3) as opool:

            # ---- small consts on the vector queue ----
            mask2 = P.tile([128, 2, 128], F32)
            nc.scalar.dma_start(out=mask2, in_=maskd.rearrange("p (h c) -> p h c", c=128))
            bqp_sb = P.tile([128, 4], F32)
            nc.scalar.dma_start(out=bqp_sb, in_=bqpd[:, :])
            bvb_sb = P.tile([128, HG, DH], F32)
            nc.scalar.dma_start(out=bvb_sb, in_=bvbd.rearrange("p (h c) -> p h c", c=DH))
            wout_sb = P.tile([128, 2, D], F32R)
            for i in range(2):
                nc.scalar.dma_start(out=wout_sb[:, i, :], in_=woutd[i * 128:(i + 1) * 128, :])

            # ---- bulk loads on the sync queue (issue order = need order) ----
            xt_sb = P.tile([128, NQC, NDT, 512], BF16)
            w_sb = P.tile([128, 6, NDT, 128], BF16)
            nc.sync.dma_start(out=w_sb[:, 0:4, :, :], in_=wd[:, 0:4, :, :])
            nc.sync.dma_start(out=xt_sb[:, 0, 0:4, :], in_=xd[:, 0, 0:4, :])
            nc.sync.dma_start(out=xt_sb[:, 0, 4:8, :], in_=xd[:, 0, 4:8, :])
            nc.sync.dma_start(out=w_sb[:, 4:6, :, :], in_=wd[:, 4:6, :, :])
            for tch in range(1, NQC):
                nc.sync.dma_start(out=xt_sb[:, tch, :, :], in_=xd[:, tch, :, :])

            # persistent activations
            qt = [P.tile([128, T], BF16, name=f"qt{p}") for p in range(2)]
            kt = [P.tile([128, T], BF16, name=f"kt{p}") for p in range(2)]
            v_aug = P.tile([128, NKT, HG * 65], BF16)
            vv = v_aug.rearrange("p k (h c) -> p k h c", c=65)
            nc.gpsimd.memset(v_aug, 1.0)  # col 64 of each 65-block stays 1.0
            ot = [P.tile([128, T], F32R, name=f"ot{p}") for p in range(2)]

            # HAM warm-up primer: dependency-free back-to-back matmuls that
            # run during the DMA/queue setup dead-zone so the PE clock is at
            # 2.4 GHz before real work starts.
            wtile = P.tile([128, 512], BF16)
            nc.vector.memset(wtile, 0.0)
            wps = ps_k.tile([128, 512], F32, tag="w", name="warm")
            for i in range(20):
                nc.tensor.matmul(
                    wps, wtile[:, 0:128], wtile, start=(i == 0), stop=(i == 19)
                )


            def qkv_chunk(qc):
                qs = slice(qc * 512, (qc + 1) * 512)
                for jt in (0, 2, 1, 3):
                    ps = ps_k.tile([128, 512], F32, tag="w", name=f"qk_{jt}_{qc}")
                    for a in range(NDT):
                        nc.tensor.matmul(
                            ps,
                            w_sb[:, jt, a, :],
                            xt_sb[:, qc, a, :],
                            start=(a == 0),
                            stop=(a == NDT - 1),
                        )
                    dst = (qt if jt < 2 else kt)[jt % 2]
                    nc.vector.tensor_scalar_add(dst[:, qs], ps, bqp_sb[:, jt:jt + 1])
                for k4 in range(4):
                    k = qc * 4 + k4
                    psv = ps_k.tile([128, GCOLS], F32, tag="w", name=f"v_{k}")
                    for a in range(NDT):
                        nc.tensor.matmul(
                            psv,
                            xt_sb[:, qc, a, k4 * 128:(k4 + 1) * 128],
                            w_sb[:, 4:6, a, :],
                            start=(a == 0),
                            stop=(a == NDT - 1),
                        )
                    nc.vector.tensor_add(
                        vv[:, k, :, 0:DH],
                        psv.rearrange("p (h c) -> p h c", c=DH),
                        bvb_sb,
                    )

            def att_chunk(qc):
                qs = slice(qc * 512, (qc + 1) * 512)
                n_kt = 4 * qc + 4
                for pr in range(2):
                    o_ps = [
                        ps_o.tile([65, 512], F32, tag="o", name=f"o_{qc}_{pr}_{hh}")
                        for hh in range(2)
                    ]
                    for k in range(n_kt):
                        j = k - 4 * qc
                        c0 = j * 128 if j >= 0 else 0
                        s = ps_s.tile([128, 2, 512], F32, tag="s", name=f"s_{qc}_{pr}_{k}")
                        for hh in range(2):
                            half = slice(hh * 64, hh * 64 + 64)
                            nc.tensor.matmul(
                                s[:, hh, c0:512],
                                kt[pr][half, k * 128:(k + 1) * 128],
                                qt[pr][half, qc * 512 + c0:(qc + 1) * 512],
                                start=True,
                                stop=True,
                            )
                        if j >= 0:
                            nc.vector.tensor_add(
                                s[:, :, j * 128:(j + 1) * 128],
                                s[:, :, j * 128:(j + 1) * 128],
                                mask2,
                            )
                        p = ppool.tile([128, 2, 512], BF16, tag="p")
                        nc.scalar.activation(
                            p[:, :, c0:512], s[:, :, c0:512], Exp, scale=0.125
                        )
                        for hh in range(2):
                            nc.tensor.matmul(
                                o_ps[hh][:, c0:512],
                                vv[:, k, 2 * pr + hh, :],
                                p[:, hh, c0:512],
                                start=(k == 0),
                                stop=(k == n_kt - 1),
                            )
                    # normalization: denominator sits in row 64 of each o tile
                    for hh in range(2):
                        half = slice(hh * 64, hh * 64 + 64)
                        rr = rpool.tile([1, 512], F32, tag="rr", name=f"rr_{qc}_{pr}_{hh}")
                        nc.vector.tensor_copy(rr, o_ps[hh][64:65, :])
                        nc.vector.reciprocal_approx_fast(out=rr, in_=rr)
                        r2 = rpool.tile([64, 512], F32, tag="r2", name=f"r2_{qc}_{pr}_{hh}")
                        nc.gpsimd.partition_broadcast(r2, rr, channels=64)
                        if qc == NQC - 1:
                            # two pieces so the tail projection can start
                            # before the whole chunk is normalized
                            for i in range(2):
                                cs = slice(qc * 512 + i * 256, qc * 512 + (i + 1) * 256)
                                nc.vector.tensor_mul(
                                    ot[pr][half, cs],
                                    o_ps[hh][0:64, i * 256:(i + 1) * 256],
                                    r2[:, i * 256:(i + 1) * 256],
                                )
                        else:
                            nc.vector.tensor_mul(ot[pr][half, qs], o_ps[hh][0:64, :], r2)

            def proj_chunk(qc):
                for tt in range(qc * 4, qc * 4 + 4):
                    ob = opool.tile([128, 2, 512], BF16, tag="ob")
                    for dc in range(2):
                        ps = ps_k.tile([128, 512], F32, tag="w", name=f"pr_{tt}_{dc}")
                        nc.tensor.matmul(
                            ps,
                            ot[0][:, tt * 128:(tt + 1) * 128],
                            wout_sb[:, 0, dc * 512:(dc + 1) * 512],
                            start=True,
                            stop=False,
                        )
                        nc.tensor.matmul(
                            ps,
                            ot[1][:, tt * 128:(tt + 1) * 128],
                            wout_sb[:, 1, dc * 512:(dc + 1) * 512],
                            start=False,
                            stop=True,
                        )
                        if qc == NQC - 1:
                            # post-exp tail: scalar is free then
                            nc.scalar.copy(ob[:, dc, :], ps)
                        else:
                            # consumed as filler during attention windows
                            # where scalar is exp-saturated: evict on DVE so
                            # the ps_k slots (and the filler reserve) flow
                            nc.vector.tensor_copy(ob[:, dc, :], ps)
                    nc.sync.dma_start(
                        out=outd[tt * 128:(tt + 1) * 128, :],
                        in_=ob.rearrange("p a c -> p (a c)"),
                    )

            # emission order = program order = scheduler priority among ready
            # instructions. att(qc) right after its qkv(qc); the next chunk's
            # QKV and all projections are emitted later, so the ready-heap
            # pulls them into exp-stall gaps on the PE instead of draining
            # them eagerly — the projections are the filler reserve for the
            # last (biggest) attention window.
            for qc in range(NQC):
                qkv_chunk(qc)
                att_chunk(qc)
            for qc in range(NQC):
                proj_chunk(qc)

    nc.compile()
    return nc


def _mask_np():
    rk = np.arange(128)[:, None]
    cq = np.arange(128)[None, :]
    return np.where(rk <= cq, 0.0, -1.0e30).astype(np.float32)


def _in_maps(x, w_qkv, b_qkv, w_out):
    bf16 = ml_dtypes.bfloat16
    mask = _mask_np()
    mask2 = np.ascontiguousarray(np.concatenate([mask, mask], axis=1))
    in_maps = []
    for c in range(8):
        b, g = divmod(c, 4)
        cols = slice(g * GCOLS, (g + 1) * GCOLS)
        wq = np.concatenate(
            [w_qkv[:, :D][:, cols], w_qkv[:, D:2 * D][:, cols], w_qkv[:, 2 * D:][:, cols]],
            axis=1,
        )
        bq = np.concatenate([b_qkv[:D][cols], b_qkv[D:2 * D][cols], b_qkv[2 * D:][cols]])
        whost = wq.reshape(NDT, 128, 6, 128).transpose(1, 2, 0, 3)
        xhost = x[b].reshape(NQC, 512, NDT, 128).transpose(3, 0, 2, 1)
        in_maps.append(
            {
                "xt": np.ascontiguousarray(xhost.astype(bf16)),
                "w": np.ascontiguousarray(whost.astype(bf16)),
                "bqp": np.ascontiguousarray(
                    bq[:2 * GCOLS].reshape(4, 128).T.astype(np.float32)
                ),
                "bvb": np.ascontiguousarray(
                    np.broadcast_to(bq[2 * GCOLS:], (128, GCOLS)).astype(np.float32)
                ),
                "wout": np.ascontiguousarray(w_out[cols, :].astype(np.float32)),
                "mask2": mask2,
            }
        )
    return in_maps


def kernel(x, w_qkv, b_qkv, w_out, b_out):
    x = np.ascontiguousarray(np.asarray(x, dtype=np.float32))
    w_qkv = np.ascontiguousarray(np.asarray(w_qkv, dtype=np.float32))
    b_qkv = np.asarray(b_qkv, dtype=np.float32)
    w_out = np.ascontiguousarray(np.asarray(w_out, dtype=np.float32))
    b_out = np.asarray(b_out, dtype=np.float32)

    if "nc" not in _CACHED:
        _CACHED["nc"] = _build()
    nc = _CACHED["nc"]

    res = run_bass_kernel_spmd(nc, _in_maps(x, w_qkv, b_qkv, w_out), list(range(8)))
    out = np.zeros((B, T, D), dtype=np.float32)
    for c in range(8):
        out[c // 4] += res.results[c]["out"].astype(np.float32)
    out += b_out
    return out
